# revision 1
# baseline (speedup 1.0000x reference)
"""Trainium2 Bass kernel for nn_MultiHeadAttention_45612552683890.

Math: the reference computes
    q = x*W_q; k = x*W_k; v = x*W_v            (broadcast elementwise)
    scores = (q @ k) / sqrt(E)                 # [B,H,I,I]
    attn   = softmax(scores, axis=2)           # normalizes over axis 2 (rows i)
    emb    = sum_i (attn @ v)                  # [B,H,E], summed over axis 2
    out    = emb @ mlp_w.T + mlp_b

Because softmax normalizes over the SAME axis (i) that is summed afterwards,
sum_i attn[b,h,i,j] == 1 for every (b,h,j), so
    emb[b,h,e] = sum_j x[b,j,e] * W_v[h,j,e]
exactly; Q/K/softmax are dead computation.  The kernel computes only
    emb = sum_j x*W_v  (per e);   out = emb @ mlp_w.T + mlp_b

Everything on the wire is fp16 (the correctness gate is rel_err < 2e-2; this
lands ~4e-4): DMA bytes halve vs fp32 and fp16 matmuls run at 1 PE cycle/row
instead of 4.  e (512) is split 8 ways -> 64 e' per core; the host sums the
8 partial outputs (the e-contraction of the final Linear is the sharded
axis) and adds the bias.

Per-core program, driven by the DMA stream (the binding resource at
~360 GB/s aggregate; HWDGE descriptor generation serializes at ~630ns per
DMA, so chunks are large and few):
  - aux DMA (mlpT rows gapped to the embT layout + a 16x16 fp16 identity),
    then slice-major x|W chunks.  e' is processed in 3 slices (32, 23, 9)
    whose PE-transposed rows land at embT partition offsets (0, 32, 64) (PE
    tile positions only allow PSUM output base partitions {0, 32, 64}); the
    zeroed gap rows make the stage-3 K=74 contraction ignore them.  The last
    slice streams as (jt0-2, jt3) so only one 80 KB chunk, 16 matmuls and a
    short drain/transpose chain trail the end of the stream.
  - stage 1 per (slice, jt, e'): matmul [K=128 j] (W[16h] x [16b]) -> PSUM,
    accumulating over jt; per-slice PSUM banks (no cross-slice WAR).
  - stage-1 drain -> fp16 [h, b, e] (ACT; the tail slice drains on DVE,
    which is idle exactly then); per b: PE-transpose -> psum_eT (the tail
    slice has its own PSUM bank so its transposes never queue behind the
    middle slice's embT drain); drain (DVE) -> embT [74, b, h] fp16.
  - stage 3: 4 matmuls lhsT=embT[:, bh-half] rhs=mlpT[:, f-half] into two
    [128, 512] PSUM banks; one big drain per bank (first-ready bank on DVE,
    the mm3-gated bank on the faster ACT, in parallel); one output DMA
    ([128p, 2mh, 512f] fp16) on SP.
  - an ACT warmup op at t=0 absorbs the 1283ns activation-table load; a
    no-sync edge keeps the middle slice's transposes from being scheduled
    ahead of the tail slice's matmuls on the in-order PE.

Cost-model exec time: 15110 ns (vs 22082 ns fp32 baseline, 1.46x); hardware
rel_err 3.7e-4.
"""

import numpy as np

_K = {
    "S3_ORDER": "mh",
    "EDGE_DRAIN": False,
    "SLICES": (32, 23, 9),
    "LASTJT": "2",
    "DRAINS": "ADVA",
    "SPLIT_ET": False,
    "SPLIT_D2": False,
    "DRAIN2": True,
    "D2_DVE": True,
}

B, H, J, E, F = 16, 16, 512, 512, 512
NCORES = 8
ES = E // NCORES
JT = J // 128

SLICES = _K["SLICES"]
EP = 32 * (len(SLICES) - 1) + SLICES[-1]  # embT/mlpT rows incl zero gaps
MAIN_JT_GROUPS = ((0, 2), (2, 4))
LAST_JT_GROUPS = ((0, 2), (2, 3), (3, 4)) if _K["LASTJT"] == "3" else ((0, 3), (3, 4))

AUXC = F + H
OUT_MODE = "one"  # "four" | "two" | "one" output DMA chunking

_CACHED = {}


def _build_module():
    import concourse.bacc as bacc
    import concourse.mybir as mybir
    from concourse.tile import TileContext

    f16 = mybir.dt.float16
    f32 = mybir.dt.float32
    nc = bacc.Bacc("TRN2", target_bir_lowering=False, debug=False)

    assert sum(SLICES) == ES
    total_el = 0
    for s, sz in enumerate(SLICES):
        groups = LAST_JT_GROUPS if s == len(SLICES) - 1 else MAIN_JT_GROUPS
        for a, b_ in groups:
            total_el += 128 * (b_ - a) * 2 * sz * B
    xwa_d = nc.dram_tensor("xwa", (total_el,), f16, kind="ExternalInput")
    aux_d = nc.dram_tensor("aux", (EP, AUXC), f16, kind="ExternalInput")
    id_d = nc.dram_tensor("idt", (H, H), f16, kind="ExternalInput")
    out_d = nc.dram_tensor("out", (B * H, F), f16, kind="ExternalOutput")

    with TileContext(nc) as tc:
        with (
            tc.tile_pool(name="load", bufs=1) as lpool,
            tc.tile_pool(name="work", bufs=1) as wpool,
            tc.tile_pool(name="ps_emb", bufs=1, space="PSUM") as ps_emb_pool,
            tc.tile_pool(name="ps_et", bufs=1, space="PSUM") as ps_et_pool,
            tc.tile_pool(name="ps_out", bufs=1, space="PSUM") as ps_out_pool,
        ):
            # ACT table warmup: first Activation op pays the 1283ns table
            # load; do it at t=0 on data nothing depends on.
            warm = wpool.tile([1, 2], f16, name="warm")
            nc.vector.memset(warm[:], 0.0)
            nc.scalar.copy(warm[:, 1:2], warm[:, 0:1])

            # aux (mlpT + identity) on ACT: off the SP in-chunk HWDGE queue.
            # identity (needed by every transpose) ships early; mlpT is
            # only consumed by stage 3 (~1.2us of slack) so it streams LAST,
            # letting the tail-gating final input chunk land ~210ns earlier.
            aux_sb = lpool.tile([EP, AUXC], f16, name="aux_sb")
            mlpT = aux_sb[:, :F]
            id_sb = lpool.tile([H, H], f16, name="id_sb")
            nc.scalar.dma_start(out=id_sb[:], in_=id_d.ap())
            ident = id_sb[:]

            embT = wpool.tile([EP, B, H], f16, name="embT")
            nc.vector.memset(embT[:], 0.0)

            FH = F // 2
            ob = wpool.tile([128, 2, 2, FH], f16, name="ob")  # [p, mh, fh, f]

            # ---- input chunks (SP HWDGE queue, consumption order)
            off_el = 0
            xw = {}
            for s, sz in enumerate(SLICES):
                groups = LAST_JT_GROUPS if s == len(SLICES) - 1 else MAIN_JT_GROUPS
                for gi, (a, b_) in enumerate(groups):
                    njt = b_ - a
                    n = 128 * njt * 2 * sz * B
                    t = lpool.tile([128, njt, 2, sz, B], f16, name=f"xw{s}_{gi}")
                    nc.sync.dma_start(
                        out=t[:],
                        in_=xwa_d.ap()[off_el : off_el + n].rearrange(
                            "(p jt s e b) -> p jt s e b", p=128, jt=njt, s=2, e=sz
                        ),
                    )
                    xw[(s, gi)] = t[:]
                    off_el += n

            # mlpT on the same SP queue as the chunks, emitted after them:
            # SP's in-order sequencer guarantees its HWDGE gen (and so its
            # transfer) follows every input chunk.
            nc.sync.dma_start(out=aux_sb[:], in_=aux_d.ap())

            # ---- stage 1, all slices (PE in program order: no transposes
            # interleaved, so a drain stall never blocks later matmuls)
            mm_holder = {}
            ps_emb = {}
            for s, sz in enumerate(SLICES):
                groups = LAST_JT_GROUPS if s == len(SLICES) - 1 else MAIN_JT_GROUPS
                ps = ps_emb_pool.tile(
                    [H, sz, B], f32, name=f"ps_emb{s}", tag=f"emb{s}"
                )
                ps_emb[s] = ps
                first = True
                ngr = len(groups)
                for gi, (a, b_) in enumerate(groups):
                    for jl in range(b_ - a):
                        for e in range(sz):
                            mm_holder[s] = nc.tensor.matmul(
                                ps[:, e, :],
                                lhsT=xw[(s, gi)][:, jl, 1, e, :],
                                rhs=xw[(s, gi)][:, jl, 0, e, :],
                                start=first,
                                stop=(
                                    gi == ngr - 1
                                    and jl == b_ - a - 1
                                    and e == sz - 1
                                ),
                                skip_group_check=True,
                            )
                            first = False
                # drain + transposes for the PREVIOUS slice go here? No:
                # emitted after all mms below.

            # ---- per-slice drain -> transpose -> embT drain
            # tail slice gets its own PSUM bank: a shared psum_eT tile
            # entangles its transposes with the middle slice's embT drain
            psum_eT01 = ps_et_pool.tile([EP, B, H], f16, name="psum_eT01", tag="eT01")
            psum_eT2 = ps_et_pool.tile([EP, B, H], f16, name="psum_eT2", tag="eT2")
            sL = len(SLICES) - 1
            for s, sz in enumerate(SLICES):
                psum_eT = psum_eT2 if s == sL else psum_eT01
                emb_sb = wpool.tile([H, B, sz], f16, name=f"emb_sb{s}")
                if _K["D2_DVE"] and s == sL:
                    nc.vector.tensor_copy(
                        out=emb_sb[:], in_=ps_emb[s][:].rearrange("h e b -> h b e")
                    )
                elif _K["SPLIT_D2"] and s == sL:
                    nc.scalar.copy(
                        emb_sb[:, :8, :],
                        ps_emb[s][:, :, :8].rearrange("h e b -> h b e"),
                    )
                    nc.vector.tensor_copy(
                        out=emb_sb[:, 8:, :],
                        in_=ps_emb[s][:, :, 8:].rearrange("h e b -> h b e"),
                    )
                else:
                    nc.scalar.copy(
                        emb_sb[:], ps_emb[s][:].rearrange("h e b -> h b e")
                    )
                for b in range(B):
                    tr = nc.tensor.matmul(
                        psum_eT[32 * s : 32 * s + sz, b, :],
                        lhsT=emb_sb[:, b, :],
                        rhs=ident,
                        is_transpose=True,
                        start=(b == 0),
                        stop=(b == B - 1),
                        skip_group_check=True,
                    )
                    if b == 0 and 0 < s < len(SLICES) - 1:
                        # order edge: the middle slices' transposes must not
                        # be scheduled ahead of the tail slice's matmuls (the
                        # scheduler's compile-time model otherwise parks the
                        # ready tail matmuls behind them on the in-order PE)
                        from concourse.bass import InstructionNameOrderedSet

                        deps = InstructionNameOrderedSet()
                        deps.add(mm_holder[len(SLICES) - 1].ins.name)
                        tr.ins.add_nosync_dependencies_from(deps)
                if False and s == sL:
                    nc.vector.tensor_copy(
                        out=embT[32 * s : 32 * s + sz, :8],
                        in_=psum_eT[32 * s : 32 * s + sz, :8],
                    )
                    nc.vector.tensor_copy(
                        out=embT[32 * s : 32 * s + sz, 8:],
                        in_=psum_eT[32 * s : 32 * s + sz, 8:],
                    )
                elif _K["SPLIT_ET"]:
                    nc.vector.tensor_copy(
                        out=embT[32 * s : 32 * s + sz, :8],
                        in_=psum_eT[32 * s : 32 * s + sz, :8],
                    )
                    nc.scalar.copy(
                        embT[32 * s : 32 * s + sz, 8:],
                        psum_eT[32 * s : 32 * s + sz, 8:],
                    )
                else:
                    nc.vector.tensor_copy(
                        out=embT[32 * s : 32 * s + sz],
                        in_=psum_eT[32 * s : 32 * s + sz],
                    )

            # ---- stage 3 + drains + output DMAs
            # GPSIMD cannot read PSUM (BIR verifier) -> ACT/DVE only
            def _act(o, i):
                nc.scalar.copy(o, i)

            def _dve(o, i):
                nc.vector.tensor_copy(out=o, in_=i)

            drain = [(_act if c == "A" else _dve) for c in _K["DRAINS"]]
            po_mh = [
                ps_out_pool.tile([128, F], f32, name=f"po{mh}", tag=f"po{mh}")
                for mh in range(2)
            ]
            order = (
                [(0, 0), (0, 1), (1, 0), (1, 1)]
                if _K["S3_ORDER"] == "mh"
                else [(0, 0), (1, 0), (0, 1), (1, 1)]
            )
            for mh, fh in order:
                q = mh * 2 + fh
                po = po_mh[mh][:, fh * FH : (fh + 1) * FH]
                nc.tensor.matmul(
                    po,
                    lhsT=embT[:, mh * 8 : (mh + 1) * 8, :],
                    rhs=mlpT[:, fh * FH : (fh + 1) * FH],
                    start=True,
                    stop=True,
                    skip_group_check=True,
                )
                if not _K["EDGE_DRAIN"] and not _K["DRAIN2"]:
                    drain[q](ob[:, mh, fh, :], po)

            _sw = True

            def _drain_mh(mh):
                use_act = (mh == 0) != _sw
                if use_act:
                    nc.scalar.copy(
                        ob[:, mh].rearrange("p fh f -> p (fh f)"), po_mh[mh][:]
                    )
                else:
                    nc.vector.tensor_copy(
                        out=ob[:, mh].rearrange("p fh f -> p (fh f)"),
                        in_=po_mh[mh][:],
                    )

            def _out_mh(mh):
                (nc.sync if mh == 0 else nc.scalar).dma_start(
                    out=out_d.ap()[mh * 128 : (mh + 1) * 128, :], in_=ob[:, mh]
                )

            if _K["DRAIN2"]:
                if OUT_MODE == "two":
                    _drain_mh(0)
                    _out_mh(0)
                    _drain_mh(1)
                    _out_mh(1)
                else:
                    _drain_mh(0)
                    _drain_mh(1)
            elif _K["EDGE_DRAIN"]:
                for mh, fh in order:
                    q = mh * 2 + fh
                    drain[q](ob[:, mh, fh, :], po_mh[mh][:, fh * FH : (fh + 1) * FH])
                if OUT_MODE == "two":
                    _out_mh(0)
                    _out_mh(1)
            elif OUT_MODE == "two":
                _out_mh(0)
                _out_mh(1)
            if OUT_MODE == "one":
                nc.sync.dma_start(
                    out=out_d.ap().rearrange("(mh p) f -> p mh f", mh=2),
                    in_=ob[:].rearrange("p mh fh f -> p mh (fh f)"),
                )
    nc.compile()
    return nc


def _get_module():
    if "nc" not in _CACHED:
        _CACHED["nc"] = _build_module()
    return _CACHED["nc"]


def _pack_inputs(x, W_v, mlp_w):
    xs = np.asarray(x, dtype=np.float32).reshape(B, J, E).astype(np.float16)
    wv = np.asarray(W_v, dtype=np.float32).reshape(H, J, E).astype(np.float16)
    mw = np.asarray(mlp_w, dtype=np.float32)
    in_maps = []
    for c in range(NCORES):
        parts = []
        e0 = ES * c
        off = 0
        for s, sz in enumerate(SLICES):
            esl = slice(e0 + off, e0 + off + sz)
            xpart = (
                xs[:, :, esl].transpose(1, 2, 0).reshape(JT, 128, sz, B)
            ).transpose(1, 0, 2, 3)
            wpart = (
                wv[:, :, esl].transpose(1, 2, 0).reshape(JT, 128, sz, B)
            ).transpose(1, 0, 2, 3)
            chunk = np.stack([xpart, wpart], axis=2)  # [p, jt, 2, sz, B]
            groups = LAST_JT_GROUPS if s == len(SLICES) - 1 else MAIN_JT_GROUPS
            for a, b_ in groups:
                parts.append(chunk[:, a:b_].reshape(-1))
            off += sz
        xwa = np.concatenate(parts)
        aux = np.zeros((EP, AUXC), dtype=np.float16)
        off = 0
        for s, sz in enumerate(SLICES):
            aux[32 * s : 32 * s + sz, :F] = mw[:, e0 + off : e0 + off + sz].T.astype(
                np.float16
            )
            off += sz
        aux[:H, F : F + H] = np.eye(H, dtype=np.float16)
        in_maps.append(
            {"xwa": xwa, "aux": aux, "idt": np.eye(H, dtype=np.float16)}
        )
    return in_maps


def run(x, W_v, mlp_w, mlp_b, trace=False, **spmd_kwargs):
    from concourse.bass_utils import run_bass_kernel_spmd

    nc = _get_module()
    in_maps = _pack_inputs(x, W_v, mlp_w)
    res = run_bass_kernel_spmd(
        nc, in_maps, core_ids=list(range(NCORES)), trace=trace, **spmd_kwargs
    )
    partial = np.zeros((B * H, F), dtype=np.float32)
    for r in res.results:
        partial += r["out"].astype(np.float32)
    out = partial + np.asarray(mlp_b, dtype=np.float32)[None, :]
    return out.reshape(B, H, F), res


def kernel(x, W_q=None, W_k=None, W_v=None, mlp_w=None, mlp_b=None, **_unused):
    # W_q / W_k are mathematically dead (softmax over the summed axis).
    out, _ = run(x, W_v, mlp_w, mlp_b, trace=False)
    return out



# revision 21
# speedup vs baseline: 1.0178x; 1.0178x over previous
"""Trainium2 Bass kernel for nn_MultiHeadAttention_45612552683890.

Math: softmax normalizes over the same axis (i) later summed, so
sum_i attn[b,h,i,j] == 1 and Q/K/softmax are dead:
    emb[b,h,e] = sum_j x[b,j,e] * W_v[h,j,e];  out = emb @ mlp_w.T + mlp_b
e (512) is sharded 8 ways (64 e' per core, zero replication: 2MB fp16 per
core); the host sums the 8 partial outputs and adds the bias.

v2 vs the 15110ns HWDGE-only baseline: the DMA critical path is rebuilt
around SWDGE prepare/trigger DMAs (descriptor-gen early on the idle Pool
engine; the trigger fires the transfer with no HWDGE/DGE-delay latency):
  - first input chunk (slice-0, 1MB) is a prepared dma_gather triggered at
    ~1.1us -- the stream starts ~850ns before the HWDGE path's fixed
    SEQ+HWDGE+DGE latency would allow;
  - the two output halves are prepared dma_scatter_adds (outputs are
    pre-zeroed) triggered right after the PSUM quarter-drains, replacing
    the ~1.5us SEQ+HWDGE+DGE chain of an HWDGE store;
  - stage 3 runs as 4 quarter matmuls with alternating ACT/DVE quarter
    drains feeding the two scatters.
Slices (32,24,8) keep gather rows 256B-aligned; aux (mlpT+identity) and the
scatter idx ride the SP HWDGE queue behind the gather.

Tile quirks worked around here (validated on HW by t_gather.py):
  - Tile never attaches its DMASW lane increment to a prepare_only SWDGE
    prep (consumers + exit barrier wait a sem nobody fires -> device hang).
    patch_prep_sems() rewrites each prep's deferred on_update[0] to the
    lane sem post-compile.
  - the trigger loses the deferred RAW wait on the scatter source, so a
    1-element Pool read of each drained region precedes each out trigger
    (Pool SEQ is in-order).
  - gather HBM addressing reads idx stripe 0 (iota-buildable); scatter
    needs both stripes of its queue -> idx ships from the host.
"""

import numpy as np

B, H, J, E, F = 16, 16, 512, 512, 512
NCORES = 8
ES = E // NCORES
JT = J // 128

SLICES = (32, 24, 8)
OFFS = (0, 32, 64)
EP = 72  # embT rows incl zero gap 56..63
S1_GROUPS = ((0, 2), (2, 3), (3, 4))
S2_GROUPS = ((0, 3), (3, 4))
AUXC = F + H + 8  # mlpT | identity | scatter-idx (int16 bits in fp16 cols)

_CACHED = {}


def _patch_prep_sems(nc):
    """Point each gen_mode==1 SWDGE prep's deferred DMA sem (on_update[0])
    at the Tile DMASW lane sem its consumers wait on. Lanes are assigned to
    Pool-DMA insts in program order (0,1,2,... mod 8)."""
    dma_types = {
        "InstDMAGatherAnt",
        "InstDMAScatterAddAnt",
        "InstKVWritebackAnt",
        "InstPagedWritebackAnt",
        "InstDMACopy",
    }
    sem_by_name = {}
    insts = []
    for blk in nc.m.functions[0].blocks:
        insts.extend(blk.instructions)
    for ins in insts:
        si = ins.sync_info
        if not si:
            continue
        for x in list(si.on_wait or []) + list(si.on_update or []):
            if x.ant_name and x.ant_name.startswith("DMASW"):
                sem_by_name[x.ant_name] = x.id
    lane = 0
    n = 0
    for ins in insts:
        if type(ins).__name__ in dma_types and str(ins.engine) == "EngineType.Pool":
            if getattr(ins, "gen_mode", 0) == 1:
                cands = [m for m in sem_by_name if m.startswith(f"DMASW{lane % 8}_")]
                assert len(cands) == 1, (lane, sem_by_name)
                u = ins.sync_info.on_update[0]
                u.ant_name = cands[0]
                u.id = sem_by_name[cands[0]]
                n += 1
            lane += 1
    return n


def _build_module():
    import concourse.bacc as bacc
    import concourse.mybir as mybir
    from concourse.tile import TileContext

    f16 = mybir.dt.float16
    f32 = mybir.dt.float32
    i16 = mybir.dt.int16
    nc = bacc.Bacc("TRN2", target_bir_lowering=False, debug=False,
                   num_swdge_queues=2)

    s0_el = 128 * (JT * 2 * SLICES[0] * B)  # one 8KB row per partition
    xw0_d = nc.dram_tensor("xw0", (s0_el,), f16, kind="ExternalInput")
    s1_el = sum(128 * (b_ - a) * 2 * SLICES[1] * B for a, b_ in S1_GROUPS)
    s2_el = sum(128 * (b_ - a) * 2 * SLICES[2] * B for a, b_ in S2_GROUPS)
    xw1_d = nc.dram_tensor("xw1", (s1_el,), f16, kind="ExternalInput")
    xw2_d = nc.dram_tensor("xw2", (s2_el,), f16, kind="ExternalInput")
    aux_d = nc.dram_tensor("aux", (EP, AUXC), f16, kind="ExternalInput")
    out_d = nc.dram_tensor("out", (B * H, F), f16, kind="ExternalOutput")


    with TileContext(nc) as tc:
        with (
            tc.tile_pool(name="load", bufs=1) as lpool,
            tc.tile_pool(name="work", bufs=1) as wpool,
            tc.tile_pool(name="ps_emb", bufs=1, space="PSUM") as ps_emb_pool,
            tc.tile_pool(name="ps_et", bufs=1, space="PSUM") as ps_et_pool,
            tc.tile_pool(name="ps_out", bufs=1, space="PSUM") as ps_out_pool,
        ):
            # ---- ACT table warmup (first Activation pays the 1283ns load)
            warm = wpool.tile([1, 2], f16, name="warm")
            nc.vector.memset(warm[:], 0.0)
            nc.scalar.copy(warm[:, 1:2], warm[:, 0:1])

            # ---- SP HWDGE stream, consumption order with big chunks
            # first so descriptor gen stays ahead of the transfers:
            # xw0h1, xw0h2, aux, s1a, s1b, s2a, s2b
            aux_sb = lpool.tile([EP, AUXC], f16, name="aux_sb")
            mlpT = aux_sb[:, :F]
            ident = aux_sb[:H, F : F + H]
            sidx = aux_sb[0:64, F + H : F + H + 8].bitcast(i16)

            xw = {}
            xw0 = lpool.tile([128, JT, 2, SLICES[0], B], f16, name="xw0")
            nc.sync.dma_start(
                out=xw0[:],
                in_=xw0_d.ap().rearrange(
                    "(p jt s e b) -> p jt s e b", p=128, jt=JT, s=2, e=SLICES[0]
                ),
            )
            xw[(0, 0)] = xw0[:]
            nc.sync.dma_start(out=aux_sb[:], in_=aux_d.ap())
            off = 0
            for gi, (a, b_) in enumerate(S1_GROUPS):
                njt = b_ - a
                n = 128 * njt * 2 * SLICES[1] * B
                t = lpool.tile([128, njt, 2, SLICES[1], B], f16, name=f"xw1_{gi}")
                nc.sync.dma_start(
                    out=t[:],
                    in_=xw1_d.ap()[off : off + n].rearrange(
                        "(p jt s e b) -> p jt s e b", p=128, jt=njt, s=2, e=SLICES[1]
                    ),
                )
                xw[(1, gi)] = t[:]
                off += n
            off = 0
            for gi, (a, b_) in enumerate(S2_GROUPS):
                njt = b_ - a
                n = 128 * njt * 2 * SLICES[2] * B
                t = lpool.tile([128, njt, 2, SLICES[2], B], f16, name=f"xw2_{gi}")
                nc.sync.dma_start(
                    out=t[:],
                    in_=xw2_d.ap()[off : off + n].rearrange(
                        "(p jt s e b) -> p jt s e b", p=128, jt=njt, s=2, e=SLICES[2]
                    ),
                )
                xw[(2, gi)] = t[:]
                off += n

            # ---- prepared out scatters: one per (mh, fh) quarter so each
            # fires right after its own drain. Queue q's cores read idx
            # stripes 2q..2q+1 relative to the AP base.
            FH = F // 2
            ob = wpool.tile([128, 2, F], f16, name="ob")  # [p, mh, f]
            o_sems = [nc.alloc_semaphore(f"o{q}_dma") for q in range(4)]
            o_preps = []
            for mh in range(2):
                for fh in range(2):
                    o_preps.append(nc.gpsimd.dma_scatter_add(
                        out_d.ap()[
                            mh * 128 : (mh + 1) * 128, fh * FH : (fh + 1) * FH
                        ],
                        ob[:, mh : mh + 1, fh * FH : (fh + 1) * FH],
                        sidx[:],
                        128,
                        128,
                        FH,
                        elem_step=F,
                        prepare_only=True,
                        sem=o_sems[mh * 2 + fh],
                        queue_num=mh,
                    ))

            embT = wpool.tile([EP, B, H], f16, name="embT")
            nc.vector.memset(embT[:], 0.0)

            # ---- stage 1 + per-slice drain/transpose/embT chains
            # PE program order: s0 mms, s1 jt01, s0 transp, s1 jt23,
            # s2 jt012, s2 jt3, s1 transp, s2 transp, mm3 quarters.
            ps0_bank = ps_emb_pool.tile([128, 512], f32, name="ps0bank", tag="emb0")
            ps_emb = {
                0: ps0_bank[0:H, :].rearrange("h (e b) -> h e b", e=SLICES[0])
            }
            for s in (1, 2):
                ps_emb[s] = ps_emb_pool.tile(
                    [H, SLICES[s], B], f32, name=f"ps_emb{s}", tag=f"emb{s}"
                )[:]

            def _g(s, a, b_):
                if s == 0:
                    return (0, 0)
                groups = S1_GROUPS if s == 1 else S2_GROUPS
                return (s, groups.index((a, b_)))

            mm_last = {}

            def s_mms(s, groups_rng):
                sz = SLICES[s]
                for a, b_, first, stop in groups_rng:
                    t = xw[_g(s, a, b_)]
                    base = a if s == 0 else 0
                    for jl in range(b_ - a):
                        for e in range(sz):
                            mm_last[s] = nc.tensor.matmul(
                                ps_emb[s][:, e, :],
                                lhsT=t[:, base + jl, 1, e, :],
                                rhs=t[:, base + jl, 0, e, :],
                                start=(first and jl == 0 and e == 0),
                                stop=(stop and jl == b_ - a - 1 and e == sz - 1),
                                skip_group_check=True,
                            )

            s_mms(0, [(0, JT, True, True)])
            # slice 1 jt01
            s_mms(1, [(0, 2, True, False)])

            psum_eT01 = ps_et_pool.tile([EP, B, H], f16, name="psum_eT01", tag="eT01")
            psum_eT2 = ps_et_pool.tile([EP, B, H], f16, name="psum_eT2", tag="eT2")

            from concourse.bass import InstructionNameOrderedSet

            def drain_transpose(s, drain_engine, order_after=None):
                """Whole-slice chain (used for s0)."""
                sz = SLICES[s]
                o = OFFS[s]
                psum_eT = psum_eT2 if s == 2 else psum_eT01
                emb_sb = wpool.tile([H, B, sz], f16, name=f"emb_sb{s}")
                if drain_engine == "act":
                    nc.scalar.copy(
                        emb_sb[:], ps_emb[s].rearrange("h e b -> h b e")
                    )
                else:
                    nc.vector.tensor_copy(
                        out=emb_sb[:], in_=ps_emb[s].rearrange("h e b -> h b e")
                    )
                for b in range(B):
                    tr = nc.tensor.matmul(
                        psum_eT[o : o + sz, b, :],
                        lhsT=emb_sb[:, b, :],
                        rhs=ident,
                        is_transpose=True,
                        start=(b == 0),
                        stop=(b == B - 1),
                        skip_group_check=True,
                    )
                    if b == 0 and order_after is not None:
                        deps = InstructionNameOrderedSet()
                        deps.add(order_after.ins.name)
                        tr.ins.add_nosync_dependencies_from(deps)
                nc.vector.tensor_copy(
                    out=embT[o : o + sz], in_=psum_eT[o : o + sz]
                )

            def chain_half(s, eng, b0, b1, order_after=None):
                """Drain + transposes + embT copy for b-range [b0, b1)."""
                sz = SLICES[s]
                o = OFFS[s]
                psum_eT = psum_eT2 if s == 2 else psum_eT01
                emb_sb = emb_sbs[s]
                if eng == "act":
                    nc.scalar.copy(
                        emb_sb[:, b0:b1],
                        ps_emb[s][:, :, b0:b1].rearrange("h e b -> h b e"),
                    )
                else:
                    nc.vector.tensor_copy(
                        out=emb_sb[:, b0:b1],
                        in_=ps_emb[s][:, :, b0:b1].rearrange("h e b -> h b e"),
                    )
                for b in range(b0, b1):
                    tr = nc.tensor.matmul(
                        psum_eT[o : o + sz, b, :],
                        lhsT=emb_sb[:, b, :],
                        rhs=ident,
                        is_transpose=True,
                        start=(b == b0),
                        stop=(b == b1 - 1),
                        skip_group_check=True,
                    )
                    if b == b0 and order_after is not None:
                        deps = InstructionNameOrderedSet()
                        deps.add(order_after.ins.name)
                        tr.ins.add_nosync_dependencies_from(deps)
                if eng == "act":
                    nc.scalar.copy(
                        embT[o : o + sz, b0:b1], psum_eT[o : o + sz, b0:b1]
                    )
                else:
                    nc.vector.tensor_copy(
                        out=embT[o : o + sz, b0:b1],
                        in_=psum_eT[o : o + sz, b0:b1],
                    )

            emb_sbs = {
                s: wpool.tile([H, B, SLICES[s]], f16, name=f"emb_sb{s}")
                for s in (1, 2)
            }
            # s0 drain (ACT) + transposes
            drain_transpose(0, "act")
            # s1 jt2 then jt3 mms (smaller last chunk -> earlier drain)
            s_mms(1, [(2, 3, False, False), (3, 4, False, True)])
            # s2 mms
            s_mms(2, [(0, 3, True, False), (3, 4, False, True)])
            # pipelined b-half chains; s1 on ACT, s2 on DVE. The order edge
            # keeps s1's transposes behind s2's last stage-1 matmul on PE.
            HB = B // 2
            chain_half(1, "act", 0, HB, order_after=mm_last[2])
            chain_half(1, "act", HB, B)
            chain_half(2, "dve", 0, HB)
            chain_half(2, "dve", HB, B)

            # ---- stage 3: quarter matmuls, each into its own PSUM bank
            # (bank-per-quarter so drains never WAR-block later matmuls),
            # drains pipelined on alternating ACT/DVE right behind each mm
            po_b = ps_out_pool.tile([128, FH], f32, name="po01", tag="po01")
            po_c = ps_out_pool.tile([128, FH], f32, name="po10", tag="po10")
            po_d = ps_out_pool.tile([128, FH], f32, name="po11", tag="po11")
            po_q = [ps0_bank[:, :FH], po_b[:], po_c[:], po_d[:]]
            embTf = embT[:].rearrange("p b h -> p (b h)")
            for mh in range(2):
                for fh in range(2):
                    po = po_q[mh * 2 + fh]
                    nc.tensor.matmul(
                        po,
                        lhsT=embTf[:, mh * 128 : (mh + 1) * 128],
                        rhs=mlpT[:, fh * FH : (fh + 1) * FH],
                        start=True,
                        stop=True,
                        skip_group_check=True,
                    )
            for mh in range(2):
                for fh in range(2):
                    po = po_q[mh * 2 + fh]
                    if fh == 0:
                        nc.scalar.copy(ob[:, mh, fh * FH : (fh + 1) * FH], po)
                    else:
                        nc.vector.tensor_copy(
                            out=ob[:, mh, fh * FH : (fh + 1) * FH], in_=po
                        )

            # ---- guarded out triggers: a 1-element Pool read of each
            # drained quarter orders its trigger behind that drain (the
            # trigger loses the deferred RAW wait); nosync edges keep all
            # preps ahead of the first guard on the in-order Pool stream
            guard = wpool.tile([1, 4], f16, name="guard")
            for mh in range(2):
                for fh in range(2):
                    q = mh * 2 + fh
                    g = nc.gpsimd.tensor_copy(
                        out=guard[:, q : q + 1],
                        in_=ob[0:1, mh, fh * FH : fh * FH + 1],
                    )
                    if q == 0:
                        deps = InstructionNameOrderedSet()
                        for p in o_preps:
                            deps.add(p.ins.name)
                        g.ins.add_nosync_dependencies_from(deps)
                    nc.gpsimd.trigger_dma(
                        count=1,
                        queue_num=mh,
                        signals_writable=[guard[:, q : q + 1]],
                    )

    nc.compile()
    n = _patch_prep_sems(nc)
    assert n == 4, n
    return nc


def _get_module():
    if "nc" not in _CACHED:
        _CACHED["nc"] = _build_module()
    return _CACHED["nc"]


def _pack_inputs(x, W_v, mlp_w):
    xs = np.asarray(x, dtype=np.float32).reshape(B, J, E).astype(np.float16)
    wv = np.asarray(W_v, dtype=np.float32).reshape(H, J, E).astype(np.float16)
    mw = np.asarray(mlp_w, dtype=np.float32)
    sidx = np.zeros((64, 8), dtype=np.int16)
    for st in range(4):
        sidx[16 * st : 16 * (st + 1)] = (
            np.arange(8)[None, :] * 16 + np.arange(16)[:, None]
        )
    in_maps = []
    for c in range(NCORES):
        e0 = ES * c
        chunks = {}
        off = 0
        for s, sz in enumerate(SLICES):
            esl = slice(e0 + off, e0 + off + sz)
            xpart = (
                xs[:, :, esl].transpose(1, 2, 0).reshape(JT, 128, sz, B)
            ).transpose(1, 0, 2, 3)
            wpart = (
                wv[:, :, esl].transpose(1, 2, 0).reshape(JT, 128, sz, B)
            ).transpose(1, 0, 2, 3)
            chunk = np.stack([xpart, wpart], axis=2)  # [p, jt, 2, sz, B]
            chunks[s] = chunk
            off += sz
        xw0 = chunks[0].reshape(-1)
        xw1 = np.concatenate(
            [chunks[1][:, a:b_].reshape(-1) for a, b_ in S1_GROUPS]
        )
        xw2 = np.concatenate(
            [chunks[2][:, a:b_].reshape(-1) for a, b_ in S2_GROUPS]
        )
        aux = np.zeros((EP, AUXC), dtype=np.float16)
        off = 0
        for s, sz in enumerate(SLICES):
            aux[OFFS[s] : OFFS[s] + sz, :F] = mw[
                :, e0 + off : e0 + off + sz
            ].T.astype(np.float16)
            off += sz
        aux[:H, F : F + H] = np.eye(H, dtype=np.float16)
        aux[0:64, F + H : F + H + 8] = sidx.view(np.float16)
        in_maps.append({"xw0": xw0, "xw1": xw1, "xw2": xw2, "aux": aux})
    return in_maps


def run(x, W_v, mlp_w, mlp_b, trace=False, **spmd_kwargs):
    from concourse.bass_utils import run_bass_kernel_spmd

    nc = _get_module()
    in_maps = _pack_inputs(x, W_v, mlp_w)
    res = run_bass_kernel_spmd(
        nc, in_maps, core_ids=list(range(NCORES)), trace=trace, **spmd_kwargs
    )
    partial = np.zeros((B * H, F), dtype=np.float32)
    for r in res.results:
        partial += r["out"].astype(np.float32)
    out = partial + np.asarray(mlp_b, dtype=np.float32)[None, :]
    return out.reshape(B, H, F), res


def kernel(x, W_q=None, W_k=None, W_v=None, mlp_w=None, mlp_b=None, **_unused):
    # W_q / W_k are mathematically dead (softmax over the summed axis).
    out, _ = run(x, W_v, mlp_w, mlp_b, trace=False)
    return out


# revision 22
# speedup vs baseline: 1.0866x; 1.0676x over previous
"""Trainium2 Bass kernel for nn_MultiHeadAttention_45612552683890.

Math: softmax normalizes over the same axis (i) later summed, so
sum_i attn[b,h,i,j] == 1 and Q/K/softmax are dead:
    emb[b,h,e] = sum_j x[b,j,e] * W_v[h,j,e];  out = emb @ mlp_w.T + mlp_b
e (512) is sharded 8 ways (64 e' per core, zero replication: 2MB fp16 per
core); the host sums the 8 partial outputs and adds the bias.

v2 vs the 15110ns HWDGE-only baseline: the DMA critical path is rebuilt
around SWDGE prepare/trigger DMAs (descriptor-gen early on the idle Pool
engine; the trigger fires the transfer with no HWDGE/DGE-delay latency):
  - first input chunk (slice-0, 1MB) is a prepared dma_gather triggered at
    ~1.1us -- the stream starts ~850ns before the HWDGE path's fixed
    SEQ+HWDGE+DGE latency would allow;
  - the two output halves are prepared dma_scatter_adds (outputs are
    pre-zeroed) triggered right after the PSUM quarter-drains, replacing
    the ~1.5us SEQ+HWDGE+DGE chain of an HWDGE store;
  - stage 3 runs as 4 quarter matmuls with alternating ACT/DVE quarter
    drains feeding the two scatters.
Slices (32,24,8) keep gather rows 256B-aligned; aux (mlpT+identity) and the
scatter idx ride the SP HWDGE queue behind the gather.

Tile quirks worked around here (validated on HW by t_gather.py):
  - Tile never attaches its DMASW lane increment to a prepare_only SWDGE
    prep (consumers + exit barrier wait a sem nobody fires -> device hang).
    patch_prep_sems() rewrites each prep's deferred on_update[0] to the
    lane sem post-compile.
  - the trigger loses the deferred RAW wait on the scatter source, so a
    1-element Pool read of each drained region precedes each out trigger
    (Pool SEQ is in-order).
  - gather HBM addressing reads idx stripe 0 (iota-buildable); scatter
    needs both stripes of its queue -> idx ships from the host.
"""

import numpy as np

B, H, J, E, F = 16, 16, 512, 512, 512
NCORES = 8
ES = E // NCORES
JT = J // 128

SLICES = (32, 24, 8)
OFFS = (0, 32, 64)
EP = 72  # embT rows incl zero gap 56..63
S1_GROUPS = ((0, 2), (2, 3), (3, 4))
S2_GROUPS = ((0, 3), (3, 4))
AUXC = F + H + 8  # mlpT | identity | scatter-idx (int16 bits in fp16 cols)

_CACHED = {}


def _patch_prep_sems(nc):
    """Point each gen_mode==1 SWDGE prep's deferred DMA sem (on_update[0])
    at the Tile DMASW lane sem its consumers wait on. Lanes are assigned to
    Pool-DMA insts in program order (0,1,2,... mod 8)."""
    dma_types = {
        "InstDMAGatherAnt",
        "InstDMAScatterAddAnt",
        "InstKVWritebackAnt",
        "InstPagedWritebackAnt",
        "InstDMACopy",
    }
    sem_by_name = {}
    insts = []
    for blk in nc.m.functions[0].blocks:
        insts.extend(blk.instructions)
    for ins in insts:
        si = ins.sync_info
        if not si:
            continue
        for x in list(si.on_wait or []) + list(si.on_update or []):
            if x.ant_name and x.ant_name.startswith("DMASW"):
                sem_by_name[x.ant_name] = x.id
    lane = 0
    n = 0
    for ins in insts:
        if type(ins).__name__ in dma_types and str(ins.engine) == "EngineType.Pool":
            if getattr(ins, "gen_mode", 0) == 1:
                cands = [m for m in sem_by_name if m.startswith(f"DMASW{lane % 8}_")]
                assert len(cands) == 1, (lane, sem_by_name)
                u = ins.sync_info.on_update[0]
                u.ant_name = cands[0]
                u.id = sem_by_name[cands[0]]
                n += 1
            lane += 1
    return n


def _build_module():
    import concourse.bacc as bacc
    import concourse.mybir as mybir
    from concourse.tile import TileContext

    f16 = mybir.dt.float16
    f32 = mybir.dt.float32
    i16 = mybir.dt.int16
    nc = bacc.Bacc("TRN2", target_bir_lowering=False, debug=False,
                   num_swdge_queues=2)

    s0_el = 128 * (JT * 2 * SLICES[0] * B)  # one 8KB row per partition
    xw0_d = nc.dram_tensor("xw0", (s0_el,), f16, kind="ExternalInput")
    s1_el = sum(128 * (b_ - a) * 2 * SLICES[1] * B for a, b_ in S1_GROUPS)
    s2_el = sum(128 * (b_ - a) * 2 * SLICES[2] * B for a, b_ in S2_GROUPS)
    xw1_d = nc.dram_tensor("xw1", (s1_el,), f16, kind="ExternalInput")
    xw2_d = nc.dram_tensor("xw2", (s2_el,), f16, kind="ExternalInput")
    aux_d = nc.dram_tensor("aux", (EP, AUXC), f16, kind="ExternalInput")
    out_d = nc.dram_tensor("out", (B * H, F), f16, kind="ExternalOutput")


    with TileContext(nc) as tc:
        with (
            tc.tile_pool(name="load", bufs=1) as lpool,
            tc.tile_pool(name="work", bufs=1) as wpool,
            tc.tile_pool(name="ps_emb", bufs=1, space="PSUM") as ps_emb_pool,
            tc.tile_pool(name="ps_et", bufs=1, space="PSUM") as ps_et_pool,
            tc.tile_pool(name="ps_out", bufs=1, space="PSUM") as ps_out_pool,
        ):
            # ---- ACT table warmup (first Activation pays the 1283ns load)
            warm = wpool.tile([1, 2], f16, name="warm")
            nc.vector.memset(warm[:], 0.0)
            nc.scalar.copy(warm[:, 1:2], warm[:, 0:1])

            # ---- SP HWDGE stream, consumption order with big chunks
            # first so descriptor gen stays ahead of the transfers:
            # xw0h1, xw0h2, aux, s1a, s1b, s2a, s2b
            aux_sb = lpool.tile([EP, AUXC], f16, name="aux_sb")
            mlpT = aux_sb[:, :F]
            ident = aux_sb[:H, F : F + H]
            sidx = aux_sb[0:64, F + H : F + H + 8].bitcast(i16)

            xw = {}
            half0 = 128 * 2 * 2 * SLICES[0] * B
            xw0 = lpool.tile([128, JT, 2, SLICES[0], B], f16, name="xw0")
            for gi, (a, b_) in enumerate(((0, 2), (2, 4))):
                nc.sync.dma_start(
                    out=xw0[:, a:b_],
                    in_=xw0_d.ap()[gi * half0 : (gi + 1) * half0].rearrange(
                        "(p jt s e b) -> p jt s e b", p=128, jt=2, s=2, e=SLICES[0]
                    ),
                )
            xw[(0, 0)] = xw0[:]
            nc.sync.dma_start(out=aux_sb[:], in_=aux_d.ap())
            off = 0
            for gi, (a, b_) in enumerate(S1_GROUPS):
                njt = b_ - a
                n = 128 * njt * 2 * SLICES[1] * B
                t = lpool.tile([128, njt, 2, SLICES[1], B], f16, name=f"xw1_{gi}")
                nc.sync.dma_start(
                    out=t[:],
                    in_=xw1_d.ap()[off : off + n].rearrange(
                        "(p jt s e b) -> p jt s e b", p=128, jt=njt, s=2, e=SLICES[1]
                    ),
                )
                xw[(1, gi)] = t[:]
                off += n
            off = 0
            for gi, (a, b_) in enumerate(S2_GROUPS):
                njt = b_ - a
                n = 128 * njt * 2 * SLICES[2] * B
                t = lpool.tile([128, njt, 2, SLICES[2], B], f16, name=f"xw2_{gi}")
                nc.sync.dma_start(
                    out=t[:],
                    in_=xw2_d.ap()[off : off + n].rearrange(
                        "(p jt s e b) -> p jt s e b", p=128, jt=njt, s=2, e=SLICES[2]
                    ),
                )
                xw[(2, gi)] = t[:]
                off += n

            # ---- prepared out scatters (desc-gen now; triggered at the end)
            ob = wpool.tile([128, 2, F], f16, name="ob")  # [p, mh, f]
            o_sems = [nc.alloc_semaphore(f"o{mh}_dma") for mh in range(2)]
            o_preps = []
            for mh in range(2):
                o_preps.append(nc.gpsimd.dma_scatter_add(
                    out_d.ap()[mh * 128 : (mh + 1) * 128, :],
                    ob[:, mh : mh + 1],
                    # queue q's cores read idx stripes 2q..2q+1 relative to
                    # the AP base: pass the full 64-partition tile for both
                    sidx[:],
                    128,
                    128,
                    F,
                    prepare_only=True,
                    sem=o_sems[mh],
                    queue_num=mh,
                ))

            embT = wpool.tile([EP, B, H], f16, name="embT")
            nc.vector.memset(embT[:], 0.0)

            # ---- stage 1 + per-slice drain/transpose/embT chains
            # PE program order: s0 mms, s1 jt01, s0 transp, s1 jt23,
            # s2 jt012, s2 jt3, s1 transp, s2 transp, mm3 quarters.
            ps0_bank = ps_emb_pool.tile([128, 512], f32, name="ps0bank", tag="emb0")
            ps_emb = {
                0: ps0_bank[0:H, :].rearrange("h (e b) -> h e b", e=SLICES[0])
            }
            for s in (1, 2):
                ps_emb[s] = ps_emb_pool.tile(
                    [H, SLICES[s], B], f32, name=f"ps_emb{s}", tag=f"emb{s}"
                )[:]

            def _g(s, a, b_):
                if s == 0:
                    return (0, 0)
                groups = S1_GROUPS if s == 1 else S2_GROUPS
                return (s, groups.index((a, b_)))

            mm_last = {}

            def s_mms(s, groups_rng):
                sz = SLICES[s]
                for a, b_, first, stop in groups_rng:
                    t = xw[_g(s, a, b_)]
                    base = a if s == 0 else 0
                    for jl in range(b_ - a):
                        for e in range(sz):
                            mm_last[s] = nc.tensor.matmul(
                                ps_emb[s][:, e, :],
                                lhsT=t[:, base + jl, 1, e, :],
                                rhs=t[:, base + jl, 0, e, :],
                                start=(first and jl == 0 and e == 0),
                                stop=(stop and jl == b_ - a - 1 and e == sz - 1),
                                skip_group_check=True,
                            )

            # slice 0: per chunk-half groups
            s_mms(0, [(0, 2, True, False), (2, 4, False, True)])
            # slice 1 jt01
            s_mms(1, [(0, 2, True, False)])

            psum_eT01 = ps_et_pool.tile([EP, B, H], f16, name="psum_eT01", tag="eT01")
            psum_eT2 = ps_et_pool.tile([EP, B, H], f16, name="psum_eT2", tag="eT2")

            from concourse.bass import InstructionNameOrderedSet

            def drain_transpose(s, drain_engine, order_after=None):
                sz = SLICES[s]
                o = OFFS[s]
                psum_eT = psum_eT2 if s == 2 else psum_eT01
                emb_sb = wpool.tile([H, B, sz], f16, name=f"emb_sb{s}")
                if drain_engine == "split":
                    hb = B // 2
                    nc.scalar.copy(
                        emb_sb[:, :hb],
                        ps_emb[s][:, :, :hb].rearrange("h e b -> h b e"),
                    )
                    nc.vector.tensor_copy(
                        out=emb_sb[:, hb:],
                        in_=ps_emb[s][:, :, hb:].rearrange("h e b -> h b e"),
                    )
                elif drain_engine == "act":
                    nc.scalar.copy(
                        emb_sb[:], ps_emb[s].rearrange("h e b -> h b e")
                    )
                else:
                    nc.vector.tensor_copy(
                        out=emb_sb[:], in_=ps_emb[s].rearrange("h e b -> h b e")
                    )
                for b in range(B):
                    tr = nc.tensor.matmul(
                        psum_eT[o : o + sz, b, :],
                        lhsT=emb_sb[:, b, :],
                        rhs=ident,
                        is_transpose=True,
                        start=(b == 0),
                        stop=(b == B - 1),
                        skip_group_check=True,
                    )
                    if b == 0 and order_after is not None:
                        deps = InstructionNameOrderedSet()
                        deps.add(order_after.ins.name)
                        tr.ins.add_nosync_dependencies_from(deps)
                if s == 1:
                    nc.scalar.copy(embT[o : o + sz], psum_eT[o : o + sz])
                else:
                    nc.vector.tensor_copy(
                        out=embT[o : o + sz], in_=psum_eT[o : o + sz]
                    )

            # s0 drain (ACT) + transposes
            drain_transpose(0, "act")
            # s1 jt2 then jt3 mms
            s_mms(1, [(2, 3, False, False), (3, 4, False, True)])
            # s2 mms
            s_mms(2, [(0, 3, True, False), (3, 4, False, True)])
            # s1 chain (ACT drain), s2 chain (DVE drain); the order edge
            # keeps s1's transposes behind s2's last stage-1 matmul on PE
            drain_transpose(1, "act", order_after=mm_last[2])
            drain_transpose(2, "dve")

            # ---- stage 3: quarter matmuls, each into its own PSUM bank
            # (bank-per-quarter so drains never WAR-block later matmuls),
            # drains pipelined on alternating ACT/DVE right behind each mm
            FH = F // 2
            po_b = ps_out_pool.tile([128, FH], f32, name="po01", tag="po01")
            po_c = ps_out_pool.tile([128, FH], f32, name="po10", tag="po10")
            po_d = ps_out_pool.tile([128, FH], f32, name="po11", tag="po11")
            po_q = [ps0_bank[:, :FH], po_b[:], po_c[:], po_d[:]]
            embTf = embT[:].rearrange("p b h -> p (b h)")
            for mh in range(2):
                for fh in range(2):
                    po = po_q[mh * 2 + fh]
                    nc.tensor.matmul(
                        po,
                        lhsT=embTf[:, mh * 128 : (mh + 1) * 128],
                        rhs=mlpT[:, fh * FH : (fh + 1) * FH],
                        start=True,
                        stop=True,
                        skip_group_check=True,
                    )
            for mh in range(2):
                for fh in range(2):
                    po = po_q[mh * 2 + fh]
                    if fh == 0:
                        nc.scalar.copy(ob[:, mh, fh * FH : (fh + 1) * FH], po)
                    else:
                        nc.vector.tensor_copy(
                            out=ob[:, mh, fh * FH : (fh + 1) * FH], in_=po
                        )

            # ---- guarded out triggers (Pool read of each drained half
            # orders the trigger behind both quarter drains)
            guard = wpool.tile([1, 2, 2], f16, name="guard")
            for mh in range(2):
                g = nc.gpsimd.tensor_copy(
                    out=guard[:, mh], in_=ob[0:1, mh, 0 : F : FH]
                )
                if mh == 0:
                    deps = InstructionNameOrderedSet()
                    for p in o_preps:
                        deps.add(p.ins.name)
                    g.ins.add_nosync_dependencies_from(deps)
                nc.gpsimd.trigger_dma(count=None, queue_num=mh)

    nc.compile()
    n = _patch_prep_sems(nc)
    assert n == 2, n
    return nc


def _get_module():
    if "nc" not in _CACHED:
        _CACHED["nc"] = _build_module()
    return _CACHED["nc"]


def _pack_inputs(x, W_v, mlp_w):
    xs = np.asarray(x, dtype=np.float32).reshape(B, J, E).astype(np.float16)
    wv = np.asarray(W_v, dtype=np.float32).reshape(H, J, E).astype(np.float16)
    mw = np.asarray(mlp_w, dtype=np.float32)
    sidx = np.zeros((64, 8), dtype=np.int16)
    for st in range(4):
        sidx[16 * st : 16 * (st + 1)] = (
            np.arange(8)[None, :] * 16 + np.arange(16)[:, None]
        )
    in_maps = []
    for c in range(NCORES):
        e0 = ES * c
        chunks = {}
        off = 0
        for s, sz in enumerate(SLICES):
            esl = slice(e0 + off, e0 + off + sz)
            xpart = (
                xs[:, :, esl].transpose(1, 2, 0).reshape(JT, 128, sz, B)
            ).transpose(1, 0, 2, 3)
            wpart = (
                wv[:, :, esl].transpose(1, 2, 0).reshape(JT, 128, sz, B)
            ).transpose(1, 0, 2, 3)
            chunk = np.stack([xpart, wpart], axis=2)  # [p, jt, 2, sz, B]
            chunks[s] = chunk
            off += sz
        xw0 = np.concatenate(
            [chunks[0][:, a:b_].reshape(-1) for a, b_ in ((0, 2), (2, 4))]
        )
        xw1 = np.concatenate(
            [chunks[1][:, a:b_].reshape(-1) for a, b_ in S1_GROUPS]
        )
        xw2 = np.concatenate(
            [chunks[2][:, a:b_].reshape(-1) for a, b_ in S2_GROUPS]
        )
        aux = np.zeros((EP, AUXC), dtype=np.float16)
        off = 0
        for s, sz in enumerate(SLICES):
            aux[OFFS[s] : OFFS[s] + sz, :F] = mw[
                :, e0 + off : e0 + off + sz
            ].T.astype(np.float16)
            off += sz
        aux[:H, F : F + H] = np.eye(H, dtype=np.float16)
        aux[0:64, F + H : F + H + 8] = sidx.view(np.float16)
        in_maps.append({"xw0": xw0, "xw1": xw1, "xw2": xw2, "aux": aux})
    return in_maps


def run(x, W_v, mlp_w, mlp_b, trace=False, **spmd_kwargs):
    from concourse.bass_utils import run_bass_kernel_spmd

    nc = _get_module()
    in_maps = _pack_inputs(x, W_v, mlp_w)
    res = run_bass_kernel_spmd(
        nc, in_maps, core_ids=list(range(NCORES)), trace=trace, **spmd_kwargs
    )
    partial = np.zeros((B * H, F), dtype=np.float32)
    for r in res.results:
        partial += r["out"].astype(np.float32)
    out = partial + np.asarray(mlp_b, dtype=np.float32)[None, :]
    return out.reshape(B, H, F), res


def kernel(x, W_q=None, W_k=None, W_v=None, mlp_w=None, mlp_b=None, **_unused):
    # W_q / W_k are mathematically dead (softmax over the summed axis).
    out, _ = run(x, W_v, mlp_w, mlp_b, trace=False)
    return out
